# revision 9
# baseline (speedup 1.0000x reference)
"""Bidirectional masked LSTM encoder on 8 Trainium2 NeuronCores.

Problem: nn_Encoder (V=32000, E=512, H=1024, B=128, T=256), f32.
  mask = (x != 0); e = emb[x]
  fwd/bwd LSTM over T with Keras mask semantics (carry h,c through masked
  steps), merge_mode='sum'. Returns (out[B,T,H], h_f+h_b, c_f+c_b).

Sharding: tensor-parallel over the 4H gate dimension. Core k owns H-rows
[128k, 128(k+1)) of every gate (i,f,g,o) for BOTH directions: a [*, 512]
column slice of W/U/b per direction (gate-interleaved order i|f|g|o).
Every core keeps the full batch B=128 in every matmul:
    z[B,512] = e_t @ W_loc + h_{t-1} @ U_loc + b_loc
with stationary operands eT/hT chunks [K=128, M=B=128] and moving operands
W/U row-chunks [128, 512] (bf16 for the e-part, float32r full-rate fp32 for
the recurrent part). After the pointwise LSTM cell update each core holds
its h-chunk; a per-step 8-way AllGather of the transposed h-chunk [128,128]
rebuilds the full hT [1024, B] every core needs for the next step. The fwd
and bwd recurrences are independent chains interleaved half-step-wise so
each direction's exchange hides under the other's compute.
"""

import sys

sys.path.insert(0, "/opt/trn_rl_repo")

import numpy as np
import ml_dtypes

import concourse.bass as bass
import concourse.bacc as bacc
import concourse.mybir as mybir
import concourse.tile as tile
from concourse.masks import make_identity
from concourse.bass import IndirectOffsetOnAxis

V, E, H = 32000, 512, 1024
B = 128
T_FULL = 256
NCORES = 8
HC = H // NCORES          # 128 h-rows per core
COLS = 4 * HC             # 512 gate columns per core per direction
KE = E // 128             # 4 contraction chunks for the e-part
KH = H // 128             # 8 contraction chunks for the h-part

F32 = mybir.dt.float32
F32R = mybir.dt.float32r
BF16 = mybir.dt.bfloat16
I32 = mybir.dt.int32

FUSED_AG = True           # one AllGather per step (both dirs) vs one per dir
LOOKAHEAD = 4             # half-steps of e-gather/transpose run-ahead
N_FILLER = 0              # dummy warm-up matmuls per half-step (HAM warmth)


def build_nc(t_steps=T_FULL, n_filler=N_FILLER):
    nc = bacc.Bacc("TRN2", target_bir_lowering=False, debug=False,
                   num_devices=NCORES)

    x_d = nc.dram_tensor("x2", [2, B, t_steps], I32, kind="ExternalInput").ap()
    emb_d = nc.dram_tensor("emb", [V, E], F32, kind="ExternalInput").ap()
    w_d = nc.dram_tensor("w", [2, E, COLS], BF16, kind="ExternalInput").ap()
    u_d = nc.dram_tensor("u", [2, H, COLS], F32R, kind="ExternalInput").ap()
    b_d = nc.dram_tensor("b", [2, 1, COLS], F32R, kind="ExternalInput").ap()
    ys_d = nc.dram_tensor("ys", [2, t_steps, B, HC], F32,
                          kind="ExternalOutput").ap()
    cfin_d = nc.dram_tensor("cfin", [2, B, HC], F32, kind="ExternalOutput").ap()

    with tile.TileContext(nc) as tc:
        with (
            tc.tile_pool(name="const", bufs=1) as cpool,
            tc.tile_pool(name="state", bufs=1) as spool,
            tc.tile_pool(name="egat", bufs=LOOKAHEAD + 3) as epool,
            tc.tile_pool(name="etile", bufs=LOOKAHEAD + 3) as etpool,
            tc.tile_pool(name="work", bufs=3) as wpool,
            tc.tile_pool(name="eps", bufs=2, space="PSUM") as etps_pool,
            tc.tile_pool(name="zps", bufs=2, space="PSUM") as zps_pool,
            tc.tile_pool(name="tps", bufs=2, space="PSUM") as tps_pool,
            tc.tile_pool(name="dram", bufs=1, space="DRAM") as dpool,
        ):
            # ---- constants / weights resident in SBUF ----
            ident = cpool.tile([128, 128], F32)
            make_identity(nc, ident[:])

            x_sb, maskf = [], []
            for d in range(2):
                xt = cpool.tile([B, t_steps], I32, name=f"xsb{d}")
                nc.sync.dma_start(out=xt[:], in_=x_d[d, :, :])
                x_sb.append(xt)
                mt = cpool.tile([B, t_steps], F32, name=f"mk{d}")
                nc.vector.tensor_scalar(
                    out=mt[:], in0=xt[:], scalar1=0, scalar2=None,
                    op0=mybir.AluOpType.not_equal,
                )
                maskf.append(mt)

            w_sb = []      # per dir: [128, KE, COLS] bf16 (chunk c at [:, c, :])
            u_sb = []      # per dir: [128, KH, COLS] f32
            b_sb = []      # per dir: [1, COLS] f32
            for d in range(2):
                wt = cpool.tile([128, KE, COLS], BF16, name=f"wsb{d}", tag=f"w{d}")
                nc.sync.dma_start(
                    out=wt[:],
                    in_=w_d[d].rearrange("(c p) n -> p c n", p=128),
                )
                w_sb.append(wt)
                ut = cpool.tile([128, KH, COLS], F32R, name=f"usb{d}", tag=f"u{d}")
                nc.sync.dma_start(
                    out=ut[:],
                    in_=u_d[d].rearrange("(c p) n -> p c n", p=128),
                )
                u_sb.append(ut)
                bt = cpool.tile([1, COLS], F32R, name=f"bsb{d}", tag=f"b{d}")
                nc.sync.dma_start(out=bt[:], in_=b_d[d, :, :])
                b_sb.append(bt)
            ones_f32 = cpool.tile([1, 128], F32)
            nc.gpsimd.memset(ones_f32[:], 1.0)
            ones_row = cpool.tile([1, 128], F32R)
            nc.vector.tensor_copy(out=ones_row[:], in_=ones_f32[:])
            zer_f32 = cpool.tile([128, KH * 128], F32)
            nc.gpsimd.memset(zer_f32[:], 0.0)

            # ---- recurrent state ----
            # hT_all[d][buf]: [128, KH*128] f32; chunk j at [:, 128j:128j+128]
            hT_all = [[spool.tile([128, KH, 128], F32R, name=f"hTa{d}{bu}", tag=f"hTa{d}{bu}")
                       for bu in range(2)] for d in range(2)]
            c_sb = [spool.tile([B, HC], F32, name=f"c{d}", tag=f"c{d}") for d in range(2)]
            h_my = [spool.tile([B, HC], F32, name=f"h{d}", tag=f"h{d}") for d in range(2)]
            for d in range(2):
                for bu in range(2):
                    nc.vector.tensor_copy(
                        out=hT_all[d][bu][:].rearrange("p c b -> p (c b)"),
                        in_=zer_f32[:])
                nc.gpsimd.memset(c_sb[d][:], 0.0)
                nc.gpsimd.memset(h_my[d][:], 0.0)

            # AllGather bounce buffers (DRAM), double-buffered per dir
            agw = 256 if FUSED_AG else 128
            nag = 1 if FUSED_AG else 2
            ag_in = [[dpool.tile([128, agw], F32R, name=f"agi{d}{bu}", tag=f"agi{d}{bu}")
                      for bu in range(2)] for d in range(nag)]
            ag_out = [[dpool.tile([NCORES * 128, agw], F32R, name=f"ago{d}{bu}", tag=f"ago{d}{bu}")
                       for bu in range(2)] for d in range(nag)]

            # filler weights: reuse u chunk 0 as dummy stationary/moving
            if n_filler:
                fill_ps = tps_pool.tile([128, 512], F32, space="PSUM",
                                        tag="fill")

            n_half = 2 * t_steps

            def step_t(s):
                d = s % 2
                k = s // 2
                return d, k, k

            # ---- e-gather + transpose pipeline (runs LOOKAHEAD ahead) ----
            et_tiles = {}
            fb_tiles = {}

            def emit_egather(s):
                if s >= n_half:
                    return
                d, t, _ = step_t(s)
                e_g = epool.tile([B, E], F32, tag="eg")
                nc.gpsimd.indirect_dma_start(
                    out=e_g[:], out_offset=None,
                    in_=emb_d[:, :],
                    in_offset=IndirectOffsetOnAxis(ap=x_sb[d][:, t:t + 1], axis=0),
                )
                eT_ps = etps_pool.tile([128, E], F32, space="PSUM", tag="etp")
                for c in range(KE):
                    nc.tensor.transpose(
                        out=eT_ps[:, c * 128:(c + 1) * 128],
                        in_=e_g[:, c * 128:(c + 1) * 128],
                        identity=ident[:],
                    )
                eT = etpool.tile([128, E], BF16, tag="et")
                nc.vector.tensor_copy(out=eT[:], in_=eT_ps[:])
                et_tiles[s] = eT

            for s in range(min(LOOKAHEAD, n_half)):
                emit_egather(s)

            # ---- main interleaved recurrence ----
            for s in range(n_half):
                d, t, k = step_t(s)
                buf, nbuf = k % 2, (k + 1) % 2
                eT = et_tiles.pop(s)

                # z = b + e@W + h@U   (PSUM accumulate)
                z_ps = zps_pool.tile([B, COLS], F32, space="PSUM", tag="z")
                nc.tensor.matmul(
                    out=z_ps[:],
                    lhsT=ones_row[:],
                    rhs=b_sb[d][:],
                    start=True, stop=False,
                )
                for c in range(KE):
                    nc.tensor.matmul(
                        out=z_ps[:],
                        lhsT=eT[:, c * 128:(c + 1) * 128],
                        rhs=w_sb[d][:, c, :],
                        start=False, stop=False,
                    )
                for j in range(KH):
                    nc.tensor.matmul(
                        out=z_ps[:],
                        lhsT=hT_all[d][buf][:, j, :],
                        rhs=u_sb[d][:, j, :],
                        start=False, stop=(j == KH - 1),
                    )

                # prefetch future e while PE is between bursts
                emit_egather(s + LOOKAHEAD)

                # pointwise LSTM cell (gate order i|f|g|o in z columns)
                sig_if = wpool.tile([B, 256], F32, tag="sif")
                nc.scalar.activation(sig_if[:], z_ps[:, 0:256],
                                     mybir.ActivationFunctionType.Sigmoid)
                tan_g = wpool.tile([B, HC], F32, tag="tg")
                nc.scalar.activation(tan_g[:], z_ps[:, 256:384],
                                     mybir.ActivationFunctionType.Tanh)
                sig_o = wpool.tile([B, HC], F32, tag="so")
                nc.scalar.activation(sig_o[:], z_ps[:, 384:512],
                                     mybir.ActivationFunctionType.Sigmoid)

                m_ap = maskf[d][:, t:t + 1]
                fc = wpool.tile([B, HC], F32, tag="fc")
                nc.vector.tensor_mul(fc[:], sig_if[:, 128:256], c_sb[d][:])
                ig = wpool.tile([B, HC], F32, tag="ig")
                nc.vector.tensor_mul(ig[:], sig_if[:, 0:128], tan_g[:])
                c_new = wpool.tile([B, HC], F32, tag="cn")
                nc.vector.tensor_add(c_new[:], fc[:], ig[:])
                tan_c = wpool.tile([B, HC], F32, tag="tc")
                nc.scalar.activation(tan_c[:], c_new[:],
                                     mybir.ActivationFunctionType.Tanh)
                h_new = wpool.tile([B, HC], F32, tag="hn")
                nc.vector.tensor_mul(h_new[:], sig_o[:], tan_c[:])

                # masked carries: y += m * (y_new - y)
                dh = wpool.tile([B, HC], F32, tag="dh")
                nc.vector.tensor_sub(dh[:], h_new[:], h_my[d][:])
                nc.vector.scalar_tensor_tensor(
                    out=h_my[d][:], in0=dh[:], scalar=m_ap, in1=h_my[d][:],
                    op0=mybir.AluOpType.mult, op1=mybir.AluOpType.add,
                )
                dc = wpool.tile([B, HC], F32, tag="dc")
                nc.vector.tensor_sub(dc[:], c_new[:], c_sb[d][:])
                nc.vector.scalar_tensor_tensor(
                    out=c_sb[d][:], in0=dc[:], scalar=m_ap, in1=c_sb[d][:],
                    op0=mybir.AluOpType.mult, op1=mybir.AluOpType.add,
                )

                # emit output row
                nc.sync.dma_start(out=ys_d[d, t], in_=h_my[d][:])

                # transpose h chunk and exchange (skip after last step per dir)
                if k < t_steps - 1:
                    hT_ps = tps_pool.tile([128, 128], F32, space="PSUM",
                                          tag="hTp")
                    nc.tensor.transpose(out=hT_ps[:], in_=h_my[d][:],
                                        identity=ident[:])
                    if FUSED_AG:
                        if d == 0:
                            hT_fb = wpool.tile([128, 256], F32R, tag="hTfb")
                            fb_tiles[k] = hT_fb
                        else:
                            hT_fb = fb_tiles.pop(k)
                        nc.vector.tensor_copy(
                            out=hT_fb[:, d * 128:(d + 1) * 128], in_=hT_ps[:])
                        if d == 1:
                            nc.gpsimd.dma_start(out=ag_in[0][nbuf][:],
                                                in_=hT_fb[:])
                            nc.gpsimd.collective_compute(
                                "AllGather",
                                mybir.AluOpType.bypass,
                                replica_groups=[list(range(NCORES))],
                                ins=[ag_in[0][nbuf].opt()],
                                outs=[ag_out[0][nbuf].opt()],
                            )
                            for dd in range(2):
                                nc.sync.dma_start(
                                    out=hT_all[dd][nbuf][:],
                                    in_=ag_out[0][nbuf][:, dd * 128:(dd + 1) * 128]
                                    .rearrange("(c p) b -> p c b", p=128),
                                )
                    else:
                        hT_sb = wpool.tile([128, 128], F32R, tag="hTs")
                        nc.vector.tensor_copy(out=hT_sb[:], in_=hT_ps[:])
                        nc.gpsimd.dma_start(out=ag_in[d][nbuf][:], in_=hT_sb[:])
                        nc.gpsimd.collective_compute(
                            "AllGather",
                            mybir.AluOpType.bypass,
                            replica_groups=[list(range(NCORES))],
                            ins=[ag_in[d][nbuf].opt()],
                            outs=[ag_out[d][nbuf].opt()],
                        )
                        nc.sync.dma_start(
                            out=hT_all[d][nbuf][:],
                            in_=ag_out[d][nbuf][:].rearrange("(c p) b -> p c b",
                                                             p=128),
                        )

                if n_filler:
                    for fi in range(n_filler):
                        nc.tensor.matmul(
                            out=fill_ps[:],
                            lhsT=hT_all[d][buf][:, 0, :],
                            rhs=u_sb[d][:, 0, :],
                            start=True, stop=True,
                        )

            for d in range(2):
                nc.sync.dma_start(out=cfin_d[d], in_=c_sb[d][:])

    nc.compile()
    return nc


def shard_inputs(x, emb, w_fwd, u_fwd, b_fwd, w_bwd, u_bwd, b_bwd):
    """Build per-core in_maps: gate-interleaved column slices per core."""
    in_maps = []
    for r in range(NCORES):
        cols = np.concatenate(
            [np.arange(g * H + r * HC, g * H + (r + 1) * HC) for g in range(4)]
        )
        w = np.stack([w_fwd[:, cols], w_bwd[:, cols]]).astype(ml_dtypes.bfloat16)
        u = np.stack([u_fwd[:, cols], u_bwd[:, cols]]).astype(np.float32)
        b = np.stack([b_fwd[cols][None, :], b_bwd[cols][None, :]]).astype(
            np.float32)
        x2 = np.stack([x, x[:, ::-1]])
        in_maps.append({
            "x2": np.ascontiguousarray(x2.astype(np.int32)),
            "emb": np.ascontiguousarray(emb.astype(np.float32)),
            "w": np.ascontiguousarray(w),
            "u": np.ascontiguousarray(u),
            "b": np.ascontiguousarray(b),
        })
    return in_maps


def assemble_outputs(results, t_steps=T_FULL):
    """results: per-core dicts with 'ys' [2,T,B,HC] and 'cfin' [2,B,HC]."""
    out = np.empty((B, t_steps, H), np.float32)
    h_fin = np.empty((B, H), np.float32)
    c_fin = np.empty((B, H), np.float32)
    for r in range(NCORES):
        ys = results[r]["ys"].reshape(2, t_steps, B, HC)
        ys_b = ys[1, ::-1]          # bwd emitted in reversed time order
        sl = slice(r * HC, (r + 1) * HC)
        out[:, :, sl] = (ys[0] + ys_b).transpose(1, 0, 2)
        h_fin[:, sl] = ys[0, t_steps - 1] + ys_b[0]
        cf = results[r]["cfin"].reshape(2, B, HC)
        c_fin[:, sl] = cf[0] + cf[1]
    return out, h_fin, c_fin


_NC_CACHE = {}


def kernel(x, emb, w_fwd, u_fwd, b_fwd, w_bwd, u_bwd, b_bwd):
    from concourse.bass_utils import run_bass_kernel_spmd

    t_steps = x.shape[1]
    if t_steps not in _NC_CACHE:
        _NC_CACHE[t_steps] = build_nc(t_steps)
    nc = _NC_CACHE[t_steps]
    in_maps = shard_inputs(x, emb, w_fwd, u_fwd, b_fwd, w_bwd, u_bwd, b_bwd)
    res = run_bass_kernel_spmd(nc, in_maps, core_ids=list(range(NCORES)))
    return assemble_outputs(res.results, t_steps)
